# revision 69
# baseline (speedup 1.0000x reference)
"""BiMamba (fwd+bwd Mamba + merge) Trainium2 Bass kernel, v3.

Sharding (8 cores): core = batch*4 + dir*2 + e_half.
Each core computes one (batch, direction) pair over 1024 of the 2048 d_inner
channels, in e-partition layout [e_p=128 x 8 tiles, t_free]. bwd cores
operate entirely in flipped time (host pre-flips x); each core returns its
full [D, L] out_proj partial (merge_w folded in) and the host sums the four
partials per batch (un-flipping the bwd pair) -- no final collective.

The sequence is processed in two pipelined 512-column halves: while the
scan phase C(h0) runs on DVE/Pool/Act, the PE computes in_proj for h1 and
the h1 AllGather (x_proj partial exchange; cheaper than AllReduce in the
collective cost model) completes in its shadow; out_proj for h0 similarly
hides under C(h1). Scan state crosses the boundary via per-plane carries
(tensor_tensor_scan initial=AP). A single f16 AllGather per half carries
dt/B/C rows; exp+ln share one activation table (see _patch_act_tables).

Engine split in C (compiler constraints: tensor_tensor_scan and
scalar_tensor_tensor are DVE-only; Pool runs tensor_tensor at 0.42 eff):
DVE does the scans, the f16-2x dBu/prod majority and the reduction tree;
Pool takes u16/gate/carry-copies and a minority of dBu/prod pairs;
Activation does all exp/ln/silu/copies.

Self-contained: hardcodes B=2, L=1024, D=1024, E=2048 (1024/core), N=16,
dt_rank=64, d_conv=4.
"""
import numpy as np

B, L, D = 2, 1024, 1024
E = 2048
EH = 1024            # channels per core (half of E)
N = 16
DTR = 64
K = 4                # d_conv
M_TILES = 8          # e-tiles per core
NB = 8               # n-plane batches
NPB = 2              # planes per batch
PL = L + 2           # plane stride with 2-col zero gap for the batched scan

_nc_cache = {}


def _patch_act_tables():
    """Narrow the activation-table chooser so Exp and Ln both resolve to the
    combined natural_log_exp_and_others set (one table load instead of a
    reload on every Exp<->Ln alternation). Set ids keep their act_info.json
    positions; only the chooser's view of exp_and_others / natural_log is
    narrowed."""
    import functools
    import concourse.hw_specs as hw_specs
    import concourse.mybir as mybir
    if getattr(hw_specs.get_activation_tables, "_bimamba_patched", False):
        return
    _orig = hw_specs.get_activation_tables

    @functools.cache
    def patched(arch):
        tabs = dict(_orig(arch))
        Ex = mybir.ActivationFunctionType.Exp
        Ln = mybir.ActivationFunctionType.Ln
        out = {}
        for name, s in tabs.items():
            if name == "exp_and_others":
                s = s - {Ex}
            elif name == "natural_log":
                s = s - {Ln}
            out[name] = s
        return out

    patched._bimamba_patched = True
    hw_specs.get_activation_tables = patched
    import sys
    bacc_mod = sys.modules.get("concourse.bacc")
    if bacc_mod is not None and hasattr(bacc_mod, "get_activation_tables"):
        bacc_mod.get_activation_tables = patched


def _build_nc():
    _patch_act_tables()
    import concourse.bacc as bacc
    import concourse.mybir as mybir
    from concourse import tile

    f32, f16 = mybir.dt.float32, mybir.dt.float16
    Alu = mybir.AluOpType
    Act = mybir.ActivationFunctionType

    nc = bacc.Bacc("TRN2", target_bir_lowering=False, debug=False, num_devices=8)

    L2 = L // 2          # sequence processed in two pipelined halves
    PL2 = L2 + 1         # plane section: 1 carry-inject col + 512 data cols

    # ---- DRAM I/O ----
    xT_d = nc.dram_tensor("xT", [D, 3 + L], f16, kind="ExternalInput")
    # pre-tiled: [p, m*1024 + kt*128 + e']  (one DMA per m-slab)
    wxiT_d = nc.dram_tensor("wxiT", [128, M_TILES * EH], f16, kind="ExternalInput")
    wzT_d = nc.dram_tensor("wzT", [128, M_TILES * EH], f16, kind="ExternalInput")
    convw_d = nc.dram_tensor("convw", [128, M_TILES * K], f32, kind="ExternalInput")
    convb_d = nc.dram_tensor("convb", [128, M_TILES], f32, kind="ExternalInput")
    xpT_d = nc.dram_tensor("xpT", [EH, 96], f16, kind="ExternalInput")
    dtwT_d = nc.dram_tensor("dtwT", [DTR, EH], f16, kind="ExternalInput")
    dtb_d = nc.dram_tensor("dtb", [128, M_TILES], f32, kind="ExternalInput")
    arate_d = nc.dram_tensor("arate", [128, M_TILES * N], f32, kind="ExternalInput")
    dp_d = nc.dram_tensor("dp", [128, M_TILES], f32, kind="ExternalInput")
    # pre-tiled: [p, dm*1024 + m*128 + d']
    woT_d = nc.dram_tensor("woT", [128, M_TILES * D], f16, kind="ExternalInput")

    dbl_in = [nc.dram_tensor(f"dbl_in{h}", [96, L2], f16, kind="Internal")
              for h in range(2)]
    dbl_gath = [nc.dram_tensor(f"dbl_gath{h}", [192, L2], f16, kind="Internal")
                for h in range(2)]
    bcrows_d = [nc.dram_tensor(f"bcrows{h}", [32, L2], f16, kind="Internal")
                for h in range(2)]
    out_d = nc.dram_tensor("out_p", [D, L], f16, kind="ExternalOutput")

    with tile.TileContext(nc) as tc:
        with tc.tile_pool(name="const", bufs=1) as cpool, \
             tc.tile_pool(name="res", bufs=1) as rpool, \
             tc.tile_pool(name="paw", bufs=1) as pwp, \
             tc.tile_pool(name="pax", bufs=2) as pxp, \
             tc.tile_pool(name="pbc", bufs=2) as pbc, \
             tc.tile_pool(name="pc", bufs=2) as pcp, \
             tc.tile_pool(name="pc1", bufs=2) as pc1, \
             tc.tile_pool(name="pcy", bufs=1) as pcy, \
             tc.tile_pool(name="pd", bufs=2) as pdp, \
             tc.tile_pool(name="psA", bufs=1, space="PSUM") as psA, \
             tc.tile_pool(name="psB", bufs=1, space="PSUM") as psB, \
             tc.tile_pool(name="psC", bufs=2, space="PSUM") as psC, \
             tc.tile_pool(name="psO", bufs=1, space="PSUM") as psO, \
             tc.tile_pool(name="psD", bufs=2, space="PSUM") as psD:
            convw = cpool.tile([128, M_TILES * K], f32)
            convb = cpool.tile([128, M_TILES], f32)
            dtb = cpool.tile([128, M_TILES], f32)
            arate = cpool.tile([128, M_TILES * N], f32)
            dp = cpool.tile([128, M_TILES], f32)
            for t_, d_ in ((convw, convw_d), (convb, convb_d), (dtb, dtb_d),
                           (arate, arate_d), (dp, dp_d)):
                nc.gpsimd.dma_start(t_[:], d_[:])

            xc16 = rpool.tile([128, M_TILES * L], f16)
            sz16 = rpool.tile([128, M_TILES * L], f16)
            g16 = rpool.tile([128, M_TILES * L], f16)
            xi16m = rpool.tile([128, M_TILES * (3 + L)], f16)
            carry = rpool.tile([128, M_TILES * N], f16)
            xT = rpool.tile([128, M_TILES * L], f16)
            for kt in range(M_TILES):
                nc.sync.dma_start(xT[:, kt * L:(kt + 1) * L],
                                  xT_d[kt * 128:(kt + 1) * 128, 3:])
            caccs = rpool.tile([128, M_TILES * L2], f16)
            zr16s = rpool.tile([128, M_TILES * L2], f16)
            junk = rpool.tile([128, 128], f16)
            nc.vector.memset(junk[:], 0.0)
            ps_junk = psO.tile([128, L2], f32)
            for _ in range(110):
                nc.tensor.matmul(ps_junk[:, 0:16], junk[:], junk[:, 0:16],
                                 start=True, stop=True)

            def emit_warm(n):
                # ready-to-run junk matmuls keep the PE p-state ramp warm
                # across dependency waits (cold matmuls run 3.7x slower)
                for _ in range(n):
                    nc.tensor.matmul(ps_junk[:], junk[:], xc16[:, 0:L2],
                                     start=True, stop=True)

            def in_proj_mm(ps, w, kslab, h):
                for kt in range(M_TILES):
                    nc.tensor.matmul(
                        ps[:],
                        w[:, kt * 128:(kt + 1) * 128],
                        xT[:, kt * L + h * L2: kt * L + (h + 1) * L2],
                        start=(kt == 0), stop=(kt == M_TILES - 1))

            def phase_a_m(m, h):
                """in_proj + conv for (m, half h) -> caccs/zr16s staging"""
                wxi = pwp.tile([128, EH], f16, tag="wxi", bufs=2)
                wz = pwp.tile([128, EH], f16, tag="wz", bufs=2)
                nc.scalar.dma_start(wxi[:], wxiT_d[:, m * EH:(m + 1) * EH])
                nc.scalar.dma_start(wz[:], wzT_d[:, m * EH:(m + 1) * EH])
                ps_xi = psA.tile([128, L2], f32, tag="xi")
                in_proj_mm(ps_xi, wxi, m, h)
                xo = m * (3 + L) + h * L2
                if h == 0:
                    nc.vector.memset(xi16m[:, xo:xo + 3], 0.0)
                nc.scalar.activation(xi16m[:, xo + 3:xo + 3 + L2], ps_xi[:], Act.Copy)
                ps_z = psA.tile([128, L2], f32, tag="z")
                in_proj_mm(ps_z, wz, m, h)
                nc.scalar.activation(zr16s[:, m * L2:(m + 1) * L2], ps_z[:], Act.Copy)
                # conv taps on DVE (f16 4x tensor_scalar + 2x adds)
                ct = pxp.tile([128, 4 * L2], f16, tag="ct")
                ct3 = ct[:].rearrange("p (k l) -> p k l", l=L2)
                for k in range(K):
                    nc.vector.tensor_scalar_mul(
                        ct3[:, k, :],
                        xi16m[:, m * (3 + L) + h * L2 + k: m * (3 + L) + h * L2 + k + L2],
                        convw[:, m * K + k:m * K + k + 1])
                c2 = pxp.tile([128, 2 * L2], f16, tag="c2")
                nc.vector.tensor_add(c2[:], ct[:, 0:2 * L2], ct[:, 2 * L2:4 * L2])
                nc.vector.tensor_add(caccs[:, m * L2:(m + 1) * L2],
                                     c2[:, 0:L2], c2[:, L2:2 * L2])

            def silu_xproj_m(m, h, ps_dbl):
                co = m * L + h * L2
                nc.scalar.activation(xc16[:, co:co + L2], caccs[:, m * L2:(m + 1) * L2],
                                     Act.Silu, bias=convb[:, m:m + 1])
                nc.scalar.activation(sz16[:, co:co + L2], zr16s[:, m * L2:(m + 1) * L2],
                                     Act.Silu)
                xp = pwp.tile([128, 96], f16, tag="xp", bufs=2)
                nc.sync.dma_start(xp[:], xpT_d[m * 128:(m + 1) * 128, :])
                nc.tensor.matmul(ps_dbl[:], xp[:],
                                 xc16[:, co:co + L2],
                                 start=(m == 0), stop=(m == M_TILES - 1))

            bca = pbc.tile([128, N * L2], f16, tag="bca", bufs=1)
            bcc = pbc.tile([128, N * L2], f16, tag="bcc", bufs=1)

            def phase_b_ag(h, ps_dbl):
                """AllGather the pair's [96, L2] partials + local add"""
                cvt16 = pbc.tile([96, L2], f16, tag="cvt", bufs=1)
                nc.vector.tensor_copy(cvt16[:], ps_dbl[:])
                nc.sync.dma_start(dbl_in[h][:], cvt16[:])
                nc.gpsimd.collective_compute(
                    "AllGather", Alu.bypass,
                    replica_groups=[[0, 1], [2, 3], [4, 5], [6, 7]],
                    ins=[dbl_in[h][:]], outs=[dbl_gath[h][:]])
                gA = pbc.tile([96, L2], f16, tag="gA", bufs=1)
                gB = pbc.tile([96, L2], f16, tag="gB", bufs=1)
                nc.sync.dma_start(gA[:], dbl_gath[h][0:96, :])
                nc.sync.dma_start(gB[:], dbl_gath[h][96:192, :])
                dbl16 = pbc.tile([96, L2], f16, tag="dbl16")
                nc.vector.tensor_add(dbl16[:], gA[:], gB[:])
                nc.sync.dma_start(bcrows_d[h][:], dbl16[64:96, :])
                return dbl16

            def phase_b_bcast(h, nbs=range(NB)):
                """B/C row broadcasts across partitions, consumer (nb) order"""
                for nb in nbs:
                    for n in (nb * NPB, nb * NPB + 1):
                        nc.sync.dma_start(bca[:, n * L2:(n + 1) * L2],
                                          bcrows_d[h][n:n + 1, :].broadcast_to([128, L2]))
                        nc.sync.dma_start(bcc[:, n * L2:(n + 1) * L2],
                                          bcrows_d[h][16 + n:17 + n, :].broadcast_to([128, L2]))

            def dt_chain(m, h, dbl16):
                dtw = pcp.tile([DTR, 128], f16, tag="dtw")
                nc.sync.dma_start(dtw[:], dtwT_d[:, m * 128:(m + 1) * 128])
                ps_dt = psC.tile([128, L2], f32, tag="dt")
                nc.tensor.matmul(ps_dt[:], dtw[:],
                                 dbl16[0:DTR, :], start=True, stop=True)
                d16 = pcp.tile([128, L2], f16, tag="d16")
                nc.scalar.activation(d16[:], ps_dt[:], Act.Exp, bias=dtb[:, m:m + 1])
                nc.scalar.activation(d16[:], d16[:], Act.Ln, bias=1.0)
                return d16

            def out_proj_dm(dm, h, wo_res=None, allow_dve=False):
                ps_o = psD.tile([128, L2], f32, tag="o")
                if wo_res is None:
                    wo = pdp.tile([128, D], f16, tag="wo")
                    nc.sync.dma_start(wo[:], woT_d[:, dm * D:(dm + 1) * D])
                    wslice = wo[:]
                else:
                    wslice = wo_res
                for m in range(M_TILES):
                    nc.tensor.matmul(ps_o[:], wslice[:, m * 128:(m + 1) * 128],
                                     g16[:, m * L + h * L2: m * L + h * L2 + L2],
                                     start=(m == 0), stop=(m == M_TILES - 1))
                finalize_ps(dm, ps_o, h, allow_dve=allow_dve)

            def finalize_ps(dm, ps, h, allow_dve=False):
                # in the final tail DVE is idle: alternate copy engine / DMA
                # queue per dm so the finalize chain isn't serialized
                ocs = pdp.tile([128, L2], f16, tag="ocs")
                if not allow_dve or dm % 2 == 0:
                    nc.scalar.activation(ocs[:], ps[:], Act.Copy)
                    nc.scalar.dma_start(out_d[dm * 128:(dm + 1) * 128,
                                              h * L2:(h + 1) * L2], ocs[:])
                else:
                    nc.vector.tensor_copy(ocs[:], ps[:])
                    nc.sync.dma_start(out_d[dm * 128:(dm + 1) * 128,
                                            h * L2:(h + 1) * L2], ocs[:])

            def phase_c_m(m, h, delta16, bca3, bcc3, extra=None):
                """scan pipeline for one (m, half); extra() emits woven work.
                Engine split: scans+tree+most dBu/prod on DVE; Pool takes the
                overflow elementwise (3.6x slower but otherwise idle)."""
                u16 = pcp.tile([128, L2], f16, tag="u16")
                nc.gpsimd.tensor_mul(u16[:], delta16[:], xc16[:, m * L + h * L2:
                                                               m * L + h * L2 + L2])
                yp16 = pcy.tile([128, N * L2], f16, tag="yp")
                yp3 = yp16[:].rearrange("p (n l) -> p n l", l=L2)
                DBU_POOL = (0, 4, 6)        # nb indices whose dBu runs on Pool
                PROD_POOL = (0, 2, 4, 6)    # nb indices whose prod runs on Pool

                def mk(nb):
                    dA = pcp.tile([128, NPB * L2], f16, tag="dA", bufs=2)
                    dBu = pcp.tile([128, NPB * L2], f16, tag="dBu", bufs=2)
                    for j in range(NPB):
                        n = nb * NPB + j
                        nc.scalar.activation(dA[:, j * L2:(j + 1) * L2], delta16[:],
                                             Act.Exp,
                                             scale=arate[:, m * N + n:m * N + n + 1])
                    eng = nc.gpsimd if nb in DBU_POOL else nc.vector
                    eng.tensor_mul(
                        dBu[:].rearrange("p (n l) -> p n l", l=L2),
                        u16[:, None, :].broadcast_to([128, NPB, L2]),
                        bca3[:, nb * NPB:(nb + 1) * NPB, :])
                    return dA, dBu

                def scan(nb, dA, dBu):
                    h4 = pcp.tile([128, NPB * L2], f16, tag="h4", bufs=3)
                    for j in range(NPB):
                        n = nb * NPB + j
                        init = 0.0 if h == 0 else carry[:, m * N + n:m * N + n + 1]
                        nc.vector.tensor_tensor_scan(
                            h4[:, j * L2:(j + 1) * L2], dA[:, j * L2:(j + 1) * L2],
                            dBu[:, j * L2:(j + 1) * L2], init, Alu.mult, Alu.add)
                    return h4

                def prod(nb, h4):
                    if h == 0:
                        # stash the boundary state for the second half
                        nc.gpsimd.tensor_copy(
                            carry[:, m * N + nb * NPB: m * N + (nb + 1) * NPB],
                            h4[:, L2 - 1::L2])
                    eng = nc.gpsimd if nb in PROD_POOL else nc.vector
                    eng.tensor_mul(yp3[:, nb * NPB:(nb + 1) * NPB, :],
                                   h4[:].rearrange("p (n l) -> p n l", l=L2),
                                   bcc3[:, nb * NPB:(nb + 1) * NPB, :])

                pend = []
                for nb in range(NB):
                    dA, dBu = mk(nb)
                    if extra is not None:
                        extra(nb)
                    pend.append((nb, scan(nb, dA, dBu)))
                    if len(pend) > 2:
                        pnb, ph4 = pend.pop(0)
                        prod(pnb, ph4)
                for pnb, ph4 in pend:
                    prod(pnb, ph4)
                y16 = pc1.tile([128, L2], f16, tag="y16", bufs=1)
                nc.vector.tensor_add(yp16[:, 0:8 * L2], yp16[:, 0:8 * L2],
                                     yp16[:, 8 * L2:16 * L2])
                nc.vector.tensor_add(yp16[:, 0:4 * L2], yp16[:, 0:4 * L2],
                                     yp16[:, 4 * L2:8 * L2])
                nc.vector.tensor_add(yp16[:, 0:2 * L2], yp16[:, 0:2 * L2],
                                     yp16[:, 2 * L2:4 * L2])
                nc.vector.tensor_add(y16[:], yp16[:, 0:L2], yp16[:, L2:2 * L2])
                # ys = xc*Dp + y as 4x tensor_scalar (DVE) + add (Pool)
                ysa = pc1.tile([128, L2], f16, tag="ysa", bufs=1)
                nc.vector.tensor_scalar_mul(ysa[:], xc16[:, m * L + h * L2:
                                                          m * L + h * L2 + L2],
                                            dp[:, m:m + 1])
                ys16 = pc1.tile([128, L2], f16, tag="ys16", bufs=1)
                nc.gpsimd.tensor_add(ys16[:], ysa[:], y16[:])
                nc.gpsimd.tensor_mul(g16[:, m * L + h * L2: m * L + h * L2 + L2],
                                     ys16[:], sz16[:, m * L + h * L2:
                                                   m * L + h * L2 + L2])

            # ================= emission =================

            # ---- A(h0) + B(h0)
            ps_dbl0 = psB.tile([96, L2], f32, tag="dbl")
            for m in range(M_TILES):
                phase_a_m(m, 0)
                silu_xproj_m(m, 0, ps_dbl0)
            dbl16_0 = phase_b_ag(0, ps_dbl0)
            phase_b_bcast(0)
            bca3 = bca[:].rearrange("p (n l) -> p n l", l=L2)
            bcc3 = bcc[:].rearrange("p (n l) -> p n l", l=L2)

            # ---- A(h1) matmuls+conv (silu/x_proj deferred to mid-C(h0))
            ps_dbl1 = psB.tile([96, L2], f32, tag="dbl")
            for m in range(M_TILES):
                phase_a_m(m, 1)

            # A(h1) was xT's last reader; reuse its SBUF as the woT cache
            for dm in range(M_TILES):
                nc.sync.dma_start(xT[:, dm * D:(dm + 1) * D],
                                  woT_d[:, dm * D:(dm + 1) * D])

            # ---- C(h0), with h1 silus/x_proj/AG woven in mid-phase
            delta16 = dt_chain(0, 0, dbl16_0)
            h1_ctx = {}
            for m in range(M_TILES):
                if m + 1 < M_TILES:
                    next_delta = dt_chain(m + 1, 0, dbl16_0)

                def extra(nb, m=m):
                    if m == 3 and nb < M_TILES:
                        # silu group for h1 (batched: two act-table switches)
                        silu_xproj_m(nb, 1, ps_dbl1)
                    if m == 4 and nb == 0:
                        h1_ctx["dbl16"] = phase_b_ag(1, ps_dbl1)
                    if m == M_TILES - 1 and nb >= 4:
                        # h1 broadcasts, slice-by-slice behind h0's last readers
                        phase_b_bcast(1, [nb - 4])
                phase_c_m(m, 0, delta16, bca3, bcc3, extra)
                if m + 1 < M_TILES:
                    delta16 = next_delta

            # ---- C(h1), with D(h0) and the dm0/1 half-1 accumulation woven
            # in; the h1 accumulators reuse the (now dead) phase-A PSUM banks
            dbl16_1 = h1_ctx["dbl16"]
            phase_b_bcast(1, range(4, NB))
            ps_h1_0 = psA.tile([128, L2], f32, tag="xi", name="ps_h1_0")
            ps_h1_1 = psA.tile([128, L2], f32, tag="z", name="ps_h1_1")

            def d_h1_partial(m):
                for i, ps in enumerate((ps_h1_0, ps_h1_1)):
                    nc.tensor.matmul(ps[:], xT[:, i * D + m * 128:
                                               i * D + (m + 1) * 128],
                                     g16[:, m * L + L2: (m + 1) * L],
                                     start=(m == 0), stop=(m == M_TILES - 1))

            delta16 = dt_chain(0, 1, dbl16_1)
            for m in range(M_TILES):
                if m + 1 < M_TILES:
                    next_delta = dt_chain(m + 1, 1, dbl16_1)

                def extra(nb, m=m):
                    if nb == 3:
                        out_proj_dm(m, 0, wo_res=xT[:, m * D:(m + 1) * D])
                    if nb == 5 and m > 0:
                        d_h1_partial(m - 1)
                    if m == M_TILES - 1 and nb >= 4:
                        emit_warm(25)
                phase_c_m(m, 1, delta16, bca3, bcc3, extra)
                if m + 1 < M_TILES:
                    delta16 = next_delta
            # ---- D(h1): open dm2/dm3 chains (m0-6 contributions are ready
            # during C(h1)'s tail) before the gate(7)-gated work blocks the
            # in-order PE queue
            def chain_open(dm):
                ps = psD.tile([128, L2], f32, tag="o")
                for m in range(M_TILES - 1):
                    nc.tensor.matmul(ps[:], xT[:, dm * D + m * 128:
                                               dm * D + (m + 1) * 128],
                                     g16[:, m * L + L2: (m + 1) * L],
                                     start=(m == 0), stop=False)
                return ps

            def chain_close(dm, ps):
                m = M_TILES - 1
                nc.tensor.matmul(ps[:], xT[:, dm * D + m * 128:
                                           dm * D + (m + 1) * 128],
                                 g16[:, m * L + L2: (m + 1) * L],
                                 start=False, stop=True)
                finalize_ps(dm, ps, 1, allow_dve=True)

            ps2 = chain_open(2)
            ps3 = chain_open(3)
            d_h1_partial(M_TILES - 1)
            finalize_ps(0, ps_h1_0, 1, allow_dve=True)
            finalize_ps(1, ps_h1_1, 1, allow_dve=True)
            chain_close(2, ps2)
            chain_close(3, ps3)
            for dm in range(4, M_TILES):
                out_proj_dm(dm, 1, wo_res=xT[:, dm * D:(dm + 1) * D],
                            allow_dve=True)

    nc.compile()
    return nc


def _host_prep(inputs):
    """Build the 8 per-core input maps from the full problem inputs."""
    x = np.asarray(inputs["x"], np.float32)
    merge_w = np.asarray(inputs["merge_w"], np.float32)
    in_maps = []
    for b in range(B):
        for di, pre in enumerate(("fwd", "bwd")):
            p = {k: np.asarray(inputs[f"{pre}_{k}"], np.float32)
                 for k in ("in_proj", "conv_w", "conv_b", "x_proj", "dt_w",
                           "dt_b", "A_log", "D", "out_proj")}
            xb = x[b]
            if di == 1:
                xb = xb[::-1]
            xTp = np.concatenate([np.zeros((D, 3), np.float32), xb.T], axis=1)
            A = -np.exp(p["A_log"])                       # (E, N)
            W = merge_w[:, di * D:(di + 1) * D] @ p["out_proj"]   # (D, E)
            def pack_lhsT(wT):
                # (D, EH) -> [p, m*1024 + kt*128 + e']
                return np.ascontiguousarray(
                    wT.reshape(M_TILES, 128, M_TILES, 128).transpose(1, 2, 0, 3)
                    .reshape(128, M_TILES * EH))

            for half in range(2):
                sl = slice(half * EH, (half + 1) * EH)
                wxiT = pack_lhsT(p["in_proj"][:E][sl].T)
                wzT = pack_lhsT(p["in_proj"][E:][sl].T)
                convw = p["conv_w"][sl].reshape(M_TILES, 128, K).transpose(1, 0, 2).reshape(128, M_TILES * K)
                convb = p["conv_b"][sl].reshape(M_TILES, 128).T
                xpT = p["x_proj"][:, sl].T                # (EH, 96)
                dtwT = p["dt_w"][sl].T                    # (DTR, EH)
                dtb = p["dt_b"][sl].reshape(M_TILES, 128).T
                arate = A[sl].reshape(M_TILES, 128, N).transpose(1, 0, 2).reshape(128, M_TILES * N)
                dpv = p["D"][sl].reshape(M_TILES, 128).T
                woT = pack_lhsT(W[:, sl].T)               # (EH, D) pre-tiled
                in_maps.append({
                    "xT": xTp.astype(np.float16),
                    "wxiT": wxiT.astype(np.float16),
                    "wzT": wzT.astype(np.float16),
                    "convw": np.ascontiguousarray(convw, np.float32),
                    "convb": np.ascontiguousarray(convb, np.float32),
                    "xpT": xpT.astype(np.float16),
                    "dtwT": dtwT.astype(np.float16),
                    "dtb": np.ascontiguousarray(dtb, np.float32),
                    "arate": np.ascontiguousarray(arate, np.float32),
                    "dp": np.ascontiguousarray(dpv, np.float32),
                    "woT": woT.astype(np.float16),
                })
    return in_maps


def _ensure_neuron_platform():
    """If a caller pinned jax to cpu, re-point it at the neuron/axon PJRT
    platform so run_bass_kernel_spmd sees the 8 NeuronCores."""
    import jax
    try:
        if len(jax.devices()) >= 8 and jax.devices()[0].platform != "cpu":
            return
    except Exception:
        pass
    for plat in ("axon", "neuron"):
        try:
            jax.config.update("jax_platforms", plat)
            if len(jax.devices()) >= 8:
                return
        except Exception:
            continue


def kernel(**inputs):
    _ensure_neuron_platform()
    from concourse.bass_utils import run_bass_kernel_spmd
    if "nc" not in _nc_cache:
        _nc_cache["nc"] = _build_nc()
    nc = _nc_cache["nc"]
    in_maps = _host_prep(inputs)
    res = run_bass_kernel_spmd(nc, in_maps, core_ids=list(range(8)))
    _nc_cache["last_results"] = res
    # Each core returns its [D, L] out_proj partial (merge_w folded in).
    # Host sums the two halves per direction, un-flips bwd, sums directions.
    out = np.zeros((B, L, D), np.float32)
    for b in range(B):
        of = (res.results[4 * b + 0]["out_p"].astype(np.float32)
              + res.results[4 * b + 1]["out_p"].astype(np.float32))
        ob = (res.results[4 * b + 2]["out_p"].astype(np.float32)
              + res.results[4 * b + 3]["out_p"].astype(np.float32))
        out[b] = (of + ob[:, ::-1]).T
    return out
